# revision 55
# baseline (speedup 1.0000x reference)
"""Trainium2 Bass kernel for causal GQA attention (B=2, T=2048, E=2048, H=16, D=128, KVH=8).

Sharding: 8 cores = 2 (batch) x 4 (head groups). Each core computes 4 query heads
(column-parallel wq) + their 2 KV heads, full causal attention for those heads, and
a partial output projection (row-parallel wo). Host sums the 4 partials per batch.

Layout strategy: everything head-transposed ([D, T] with D on partitions) so that
no on-chip transposes are needed anywhere:
  - qT/kT = wq/wk.T @ x.T directly from PE (lhsT = weight slice, rhs = x.T)
  - scores S^T[k, q] = kT.T @ qT (lhsT = kT block, rhs = qT chunk)
  - attn_outT[d, q] = v_nat.T @ expS^T (lhsT = v natural [t, d], rhs = exp block)
  - out[t, e] = attn_outT.T @ wo (lhsT = attn_outT, rhs = wo rows)
RoPE pairs are de-interleaved by permuting wq/wk columns on the host (scores are
invariant since q and k use the same permutation), so rope becomes a half-swap.
Softmax is computed without max-subtraction (logits are O(5)); causal masking is a
0/1 multiply on the exp'd diagonal blocks, full blocks above the diagonal skipped.
Denominators via ones-vector matmuls accumulated in PSUM alongside the AV matmuls.

`reps`: wraps the whole body (including input DMA) in an on-device For_i loop —
used only for latency-slope timing in test.py; the graded path uses reps=1.
"""

import numpy as np
import ml_dtypes

BF16 = ml_dtypes.bfloat16

B, T, E = 2, 2048, 2048
H, D = 16, 128
KVH = 8
THETA = 10000.0
P = 128
EO = E // P          # 16 contraction chunks
CH = 512             # q-chunk width
NTQ = T // CH        # 4 q chunks
NTB = T // P         # 16 t blocks
NH = H // 4          # 4 q heads per core
NKV = 2              # kv heads per core
SCALE = float(D) ** -0.5

_NC_CACHE = {}
_PHASE_LIMIT = "full"   # "proj" | "attn" | "full" — for phase-cost probing only
_K_STREAM = False       # False: chunk-major K projection (no bank cycling)
_EARLY_ATTN = False     # True: open the first attention block before Q's last chunk
_SROW_BATCH = True      # True: denominator matmuls as one consecutive pass per
                        # (head, chunk) instead of interleaved with AV (fewer
                        # concurrently-open PSUM accumulation groups)


def _build_nc(reps=1):
    import concourse.mybir as mybir
    import concourse.tile as tile
    from concourse import bacc

    nc = bacc.Bacc(None, target_bir_lowering=False)
    dt = mybir.dt
    f32, bf16 = dt.float32, dt.bfloat16
    Exp = mybir.ActivationFunctionType.Exp

    xT_d = nc.dram_tensor("xT", [E, T], bf16, kind="ExternalInput")
    wq_d = nc.dram_tensor("wq", [E, NH * D], bf16, kind="ExternalInput")
    wk_d = nc.dram_tensor("wk", [E, NKV * D], bf16, kind="ExternalInput")
    wv_d = nc.dram_tensor("wv", [E, NKV * D], bf16, kind="ExternalInput")
    wo_d = nc.dram_tensor("wo", [NH * D, E], bf16, kind="ExternalInput")
    cos_d = nc.dram_tensor("cosd", [P, T], bf16, kind="ExternalInput")
    sin_d = nc.dram_tensor("sind", [P, T], bf16, kind="ExternalInput")
    mk_d = nc.dram_tensor("mkd", [4, P, CH], bf16, kind="ExternalInput")
    o_d = nc.dram_tensor("od", [T, E], bf16, kind="ExternalOutput")

    xT_r = xT_d.rearrange("(eo p) t -> p eo t", p=P)
    wq_r = wq_d.rearrange("(eo p) m -> p eo m", p=P)
    wk_r = wk_d.rearrange("(eo p) m -> p eo m", p=P)
    wv_r = wv_d.rearrange("(eo p) m -> p eo m", p=P)
    wo_r = wo_d.rearrange("(h p) e -> p h e", p=P)
    o_r = o_d.rearrange("(tb p) e -> p tb e", p=P)

    with tile.TileContext(nc) as tc:
        with (
            tc.tile_pool(name="singles", bufs=1) as sg,
            tc.tile_pool(name="ropet", bufs=2) as rp,
            tc.tile_pool(name="expp", bufs=18) as ep,
            tc.tile_pool(name="normp", bufs=2) as np_,
            tc.tile_pool(name="outst", bufs=2) as op_,
        ):

            def emit_body():
                # tiles (allocation only; DMA issue order below is what matters)
                wk_sb = sg.tile([P, EO, NKV * D], bf16, name="wk_sb", tag="wk_sb")
                xT_sb = sg.tile([P, EO, T], bf16, name="xT_sb", tag="xT_sb")
                wv_sb = sg.tile([P, EO, NKV * D], bf16, name="wv_sb", tag="wv_sb")
                wq_sb = sg.tile([P, EO, NH * D], bf16, name="wq_sb", tag="wq_sb")
                cos_sb = sg.tile([P, T], bf16, name="cos_sb", tag="cos_sb")
                sin_sb = sg.tile([P, T], bf16, name="sin_sb", tag="sin_sb")
                wo_sb = sg.tile([P, NH, E], bf16, name="wo_sb", tag="wo_sb")
                mk_sb = [sg.tile([P, CH], bf16, name=f"mk{i}", tag=f"mk{i}") for i in range(4)]

                # Two parallel DMA queues, issue order = consumption order.
                # SP queue: the 8MB xT stream (eo 0 split for fast first data),
                # then late-need weights. ACT queue: small early-need weights,
                # so wk rides alongside the first xT chunks.
                # wk rides the ACT queue (first quarter alone so the first K
                # matmul starts ~0.5us in); everything else queues on SP behind
                # the xT stream in consumption order, keeping the xT cadence
                # (~1.4us/chunk) under the K-stream PE rate (~1.7us/chunk).
                nc.scalar.dma_start(wk_sb[:, 0, :], wk_r[:, 0, :])
                nc.scalar.dma_start(wk_sb[:, 1:4, :], wk_r[:, 1:4, :])
                nc.scalar.dma_start(wk_sb[:, 4:10, :], wk_r[:, 4:10, :])
                nc.scalar.dma_start(wk_sb[:, 10:16, :], wk_r[:, 10:16, :])
                for s in range(2):
                    nc.sync.dma_start(xT_sb[:, 0, CH * 2 * s:CH * 2 * (s + 1)],
                                      xT_r[:, 0, CH * 2 * s:CH * 2 * (s + 1)])
                for eo in range(1, EO):
                    nc.sync.dma_start(xT_sb[:, eo, :], xT_r[:, eo, :])
                nc.sync.dma_start(wv_sb[:], wv_r[:])
                nc.sync.dma_start(cos_sb[:], cos_d[:])
                nc.sync.dma_start(sin_sb[:], sin_d[:])
                nc.sync.dma_start(wq_sb[:], wq_r[:])
                for i in range(4):
                    nc.sync.dma_start(mk_sb[i][:], mk_d[i])
                nc.sync.dma_start(wo_sb[:], wo_r[:])

                ones_sb = sg.tile([P, 1], bf16, name="ones_sb", tag="ones_sb")
                nc.vector.memset(ones_sb[:], 1.0)

                kT_sb = [sg.tile([P, T], bf16, name=f"kT{g}", tag=f"kT{g}") for g in range(NKV)]
                qT_sb = [sg.tile([P, T], bf16, name=f"qT{h}", tag=f"qT{h}") for h in range(NH)]
                v_sb = sg.tile([P, NTB, NKV * D], bf16, name="v_sb", tag="v_sb")

                def rope_chunk(dest, sl, ps):
                    # dest[:, sl] = ps * cos + swap_halves(ps) * sin (sin rows 0:64
                    # pre-negated). ACT builds the half-swapped copy from PSUM
                    # (PSUM sources may cross partition starts; SBUF-SBUF ops may
                    # not), so the remaining DVE muls run all-SBUF 2-byte at the
                    # fast DVE rate instead of half-partition-height full-cost ops.
                    xs = rp.tile([P, CH], bf16, name="ropexs", tag="ropexs")
                    nc.scalar.copy(xs[0:64, :], ps[64:128, :])
                    nc.scalar.copy(xs[64:128, :], ps[0:64, :])
                    t1 = rp.tile([P, CH], bf16, name="ropet1", tag="ropet1")
                    nc.vector.tensor_mul(t1[:], ps[:], cos_sb[:, sl])
                    t2 = rp.tile([P, CH], bf16, name="ropet2", tag="ropet2")
                    nc.vector.tensor_mul(t2[:], xs[:], sin_sb[:, sl])
                    nc.vector.tensor_add(dest[:, sl], t1[:], t2[:])

                # Projection phase: one 8-bank PSUM ring pool for K, V and Q so
                # bank reuse follows consumption order deterministically.
                # K streams the xT chunks as the DMA delivers them: all 8
                # (kv-head, q-chunk) accumulators live in PSUM at once, one
                # matmul per accumulator per arriving eo chunk (~1.7us PE work
                # per ~1.4us DMA cadence), instead of stalling ~20us for the
                # full xT stream as with chunk-major order. V and Q then run
                # at full PE speed on the resident xT; each ring slot's next
                # user only waits for that slot's drain (ACT rope copy / v
                # copy), never for the whole previous stage.
                with tc.tile_pool(name="pp", bufs=8, space="PSUM") as pp:
                    if _K_STREAM:
                        psk = [[pp.tile([P, CH], f32, name=f"psk{g}_{t}", tag="pp")
                                for t in range(NTQ)] for g in range(NKV)]
                        for eo in range(EO):
                            for g in range(NKV):
                                for tci in range(NTQ):
                                    nc.tensor.matmul(
                                        psk[g][tci][:],
                                        wk_sb[:, eo, D * g:D * (g + 1)],
                                        xT_sb[:, eo, CH * tci:CH * (tci + 1)],
                                        start=(eo == 0), stop=(eo == EO - 1),
                                    )
                        for g in range(NKV):
                            for tci in range(NTQ):
                                rope_chunk(kT_sb[g], slice(CH * tci, CH * (tci + 1)),
                                           psk[g][tci])
                    else:
                        for g in range(NKV):
                            for tci in range(NTQ):
                                psk1 = pp.tile([P, CH], f32, name=f"psk{g}_{tci}", tag="pp")
                                for eo in range(EO):
                                    nc.tensor.matmul(
                                        psk1[:],
                                        wk_sb[:, eo, D * g:D * (g + 1)],
                                        xT_sb[:, eo, CH * tci:CH * (tci + 1)],
                                        start=(eo == 0), stop=(eo == EO - 1),
                                    )
                                rope_chunk(kT_sb[g], slice(CH * tci, CH * (tci + 1)), psk1)

                    for u in range(8):
                        psv = pp.tile([P, CH], f32, name="psv", tag="pp")
                        for k2 in range(2):
                            tb = 2 * u + k2
                            for eo in range(EO):
                                nc.tensor.matmul(
                                    psv[:, 256 * k2:256 * (k2 + 1)],
                                    xT_sb[:, eo, P * tb:P * (tb + 1)],
                                    wv_sb[:, eo, :],
                                    start=(eo == 0), stop=(eo == EO - 1),
                                )
                        for k2 in range(2):
                            nc.vector.tensor_copy(out=v_sb[:, 2 * u + k2, :], in_=psv[:, 256 * k2:256 * (k2 + 1)])

                with (
                    tc.tile_pool(name="pj", bufs=2, space="PSUM") as pj,
                    tc.tile_pool(name="ps_s", bufs=3, space="PSUM") as ps_s,
                    tc.tile_pool(name="ps_o", bufs=2, space="PSUM") as ps_o,
                    tc.tile_pool(name="ps_m", bufs=1, space="PSUM") as ps_m,
                ):
                    def attn(h, tci, norm_pieces=1):
                        # generator: yields once per k-block so the driver can
                        # interleave wo-projection steps between blocks
                        g = h // 2
                        sl = slice(CH * tci, CH * (tci + 1))
                        ntk = 4 * tci + 4
                        o_ps = ps_o.tile([P, CH], f32, name="o_ps", tag="o_ps")
                        s_row = ps_m.tile([1, CH], f32, name="s_row", tag="s_row")

                        def block_c0(j):
                            di = j - 4 * tci
                            return P * di if di > 0 else 0

                        def scores_exp(j):
                            # scores block j + its exp, emitted one block ahead of
                            # the consuming AV/denominator matmuls so PE never
                            # waits on ACT's exp latency
                            c0 = block_c0(j)
                            qsl = slice(CH * tci + c0, CH * (tci + 1))
                            s_ps = ps_s.tile([P, CH], f32, name="s_ps", tag="s_ps")
                            nc.tensor.matmul(
                                s_ps[:, c0:], kT_sb[g][:, P * j:P * (j + 1)], qT_sb[h][:, qsl],
                                start=True, stop=True,
                            )
                            e_t = ep.tile([P, CH], bf16, name="e_t", tag="e_t")
                            nc.scalar.activation(e_t[:, c0:], s_ps[:, c0:], Exp, scale=SCALE)
                            return e_t

                        # scores/exp run a quad ahead of the AV pass so the o_ps
                        # accumulation group gets 4 consecutive same-bank matmuls
                        # between s_ps singles (open-group interleaving is
                        # expensive on hardware), and the denominator matmuls
                        # run as one consecutive pass at the end.
                        pend = [scores_exp(j) for j in range(min(4, ntk))]
                        kept = []
                        for jq in range(0, ntk, 4):
                            for dj in range(4):
                                j = jq + dj
                                e_t = pend.pop(0)
                                c0 = block_c0(j)
                                if j - 4 * tci >= 0:
                                    nc.vector.tensor_mul(e_t[:, c0:], e_t[:, c0:],
                                                         mk_sb[j - 4 * tci][:, c0:])
                                nc.tensor.matmul(
                                    o_ps[:, c0:], v_sb[:, j, D * g:D * (g + 1)], e_t[:, c0:],
                                    start=(j == 0), stop=(j == ntk - 1),
                                )
                                if _SROW_BATCH:
                                    kept.append((e_t, c0))
                                else:
                                    nc.tensor.matmul(
                                        s_row[:, c0:], ones_sb[:], e_t[:, c0:],
                                        start=(j == 0), stop=(j == ntk - 1),
                                    )
                                yield
                            for dj in range(4):
                                if jq + 4 + dj < ntk:
                                    pend.append(scores_exp(jq + 4 + dj))
                        for j, (e_t, c0) in enumerate(kept):
                            nc.tensor.matmul(
                                s_row[:, c0:], ones_sb[:], e_t[:, c0:],
                                start=(j == 0), stop=(j == ntk - 1),
                            )
                        # norm_pieces>1 (very last head): normalize in t-block-sized
                        # pieces so the trailing wo strips can start on piece 0
                        # instead of waiting for the full-width chain
                        w = CH // norm_pieces
                        for pc in range(norm_pieces):
                            psl = slice(w * pc, w * (pc + 1))
                            rec = np_.tile([1, CH], f32, name="rec", tag="rec")
                            nc.vector.reciprocal(rec[:, psl], s_row[:, psl])
                            bc = np_.tile([P, CH], f32, name="bc", tag="bc")
                            nc.gpsimd.partition_broadcast(bc[:, psl], rec[:, psl])
                            nc.vector.tensor_mul(qT_sb[h][:, sl][:, psl], o_ps[:, psl], bc[:, psl])

                    if _PHASE_LIMIT == "proj":
                        for h in range(NH):
                            nc.sync.dma_start(o_r[:, 4 * h, :], qT_sb[h][:])
                        for g in range(NKV):
                            nc.sync.dma_start(o_r[:, 8 + g, :], kT_sb[g][:])
                        return

                    # The partial output projection for each chunk is interleaved
                    # into the NEXT chunk's attention at k-block granularity (one
                    # wo strip per attention block): PE alternates ~640ns
                    # attention work and ~850ns wo work while ACT runs exps
                    # ahead into the e_t ring, so neither engine stalls the
                    # other at chunk transitions.
                    def wo_group(tci, split=False):
                        # generator: yields once per 512-col strip.
                        # split=True (final t-block): DMA each 512-col strip as
                        # its copy lands so the end-of-kernel drain is one strip,
                        # not a full row
                        for tb in range(4 * tci, 4 * tci + 4):
                            ost = op_.tile([P, E], bf16, name="ost", tag="ost")
                            strips = split and tb == 4 * tci + 3
                            for n in range(4):
                                wop = pj.tile([P, CH], f32, name="wop", tag="pj")
                                for h in range(NH):
                                    nc.tensor.matmul(
                                        wop[:],
                                        qT_sb[h][:, P * tb:P * (tb + 1)],
                                        wo_sb[:, h, CH * n:CH * (n + 1)],
                                        start=(h == 0), stop=(h == NH - 1),
                                    )
                                # wo copies on DVE mid-attention (ACT is saturated
                                # by exps there; gpsimd cannot read PSUM); the
                                # final group goes to ACT, idle once exps end,
                                # while DVE still drains normalize work
                                eng = nc.scalar if split else nc.vector
                                if split:
                                    eng.copy(ost[:, CH * n:CH * (n + 1)], wop[:])
                                else:
                                    eng.tensor_copy(out=ost[:, CH * n:CH * (n + 1)], in_=wop[:])
                                if strips:
                                    nc.sync.dma_start(o_r[:, tb, CH * n:CH * (n + 1)],
                                                      ost[:, CH * n:CH * (n + 1)])
                                yield
                            if not strips:
                                nc.sync.dma_start(o_r[:, tb, :], ost[:])

                    if _PHASE_LIMIT == "attn":
                        for tci in range(NTQ):
                            for h in range(NH):
                                for _ in attn(h, tci):
                                    pass
                        for h in range(NH):
                            nc.sync.dma_start(o_r[:, 4 * h, :], qT_sb[h][:])
                        return

                    # Q projection on pj's 2-buf ring (chunk-major, eo-inner) so
                    # the attention banks (ps_s/o/m) carry no WAR against late Q
                    # rope drains when attention starts. The first attention
                    # generator's opening block is emitted just before the last
                    # Q chunk so its exp latency hides under those matmuls.
                    first_gen = None
                    for i, (h, tci) in enumerate([(h, t) for h in range(NH) for t in range(NTQ)]):
                        if _EARLY_ATTN and i == NH * NTQ - 1:
                            first_gen = attn(0, 0)
                            next(first_gen)
                        psq = pj.tile([P, CH], f32, name=f"psq{tci}", tag="pj")
                        for eo in range(EO):
                            nc.tensor.matmul(
                                psq[:],
                                wq_sb[:, eo, D * h:D * (h + 1)],
                                xT_sb[:, eo, CH * tci:CH * (tci + 1)],
                                start=(eo == 0), stop=(eo == EO - 1),
                            )
                        rope_chunk(qT_sb[h], slice(CH * tci, CH * (tci + 1)), psq)

                    done = object()
                    wo_pend = None
                    stagger = 0
                    for tci in range(NTQ):
                        if tci > 0:
                            wo_pend = wo_group(tci - 1)
                            # hold the first wo steps back a couple of blocks so
                            # the previous chunk's final normalize chain drains
                            stagger = 2
                        for h in range(NH):
                            last = tci == NTQ - 1 and h == NH - 1
                            gen = first_gen if (tci == 0 and h == 0 and first_gen is not None) else \
                                attn(h, tci, norm_pieces=4 if last else 1)
                            for _ in gen:
                                if stagger > 0:
                                    stagger -= 1
                                elif wo_pend is not None and next(wo_pend, done) is done:
                                    wo_pend = None
                    for _ in wo_group(NTQ - 1, split=True):
                        pass

            if reps > 1:
                with tc.For_i(0, reps, 1):
                    emit_body()
            else:
                emit_body()

    nc.finalize()
    return nc


def get_nc(reps=1):
    if reps not in _NC_CACHE:
        _NC_CACHE[reps] = _build_nc(reps)
    return _NC_CACHE[reps]


def make_host_inputs(x, wq, wk, wv, wo):
    """Returns per-core in_maps (list of 8 dicts)."""
    perm = np.concatenate([np.arange(0, D, 2), np.arange(1, D, 2)])
    wq4 = np.asarray(wq).reshape(E, H, D)[:, :, perm]
    wk4 = np.asarray(wk).reshape(E, KVH, D)[:, :, perm]
    wv4 = np.asarray(wv).reshape(E, KVH, D)
    wo4 = np.asarray(wo).reshape(H, D, E)
    xT = np.ascontiguousarray(np.transpose(np.asarray(x), (0, 2, 1))).astype(BF16)

    # mirror reference's float32 rope computation
    invf = 1.0 / (np.float32(THETA) ** (np.arange(0, D, 2, dtype=np.float32) / np.float32(D)))
    ang = np.arange(T, dtype=np.float32)[None, :] * invf[:, None]     # [64, T]
    cosv = np.cos(ang).astype(np.float32)
    sinv = np.sin(ang).astype(np.float32)
    cos_h = np.concatenate([cosv, cosv], 0).astype(BF16)
    sin_h = np.concatenate([-sinv, sinv], 0).astype(BF16)

    ii = np.arange(P)[:, None]
    jj = np.arange(CH)[None, :]
    mk_h = np.stack([(jj >= ii + P * di) for di in range(4)]).astype(BF16)

    in_maps = []
    for c in range(8):
        b, hg = divmod(c, 4)
        qs = slice(4 * hg, 4 * hg + 4)
        ks = slice(2 * hg, 2 * hg + 2)
        in_maps.append({
            "xT": xT[b],
            "wq": np.ascontiguousarray(wq4[:, qs].reshape(E, NH * D)).astype(BF16),
            "wk": np.ascontiguousarray(wk4[:, ks].reshape(E, NKV * D)).astype(BF16),
            "wv": np.ascontiguousarray(wv4[:, ks].reshape(E, NKV * D)).astype(BF16),
            "wo": np.ascontiguousarray(wo4[qs].reshape(NH * D, E)).astype(BF16),
            "cosd": cos_h,
            "sind": sin_h,
            "mkd": mk_h,
        })
    return in_maps


def kernel(x, mask, wq, wk, wv, wo, **extra):
    from concourse.bass_utils import run_bass_kernel_spmd

    nc = get_nc()
    in_maps = make_host_inputs(x, wq, wk, wv, wo)
    res = run_bass_kernel_spmd(nc, in_maps, core_ids=list(range(8)))
    out = np.zeros((B, T, E), np.float32)
    for c in range(8):
        out[c // 4] += res.results[c]["od"].astype(np.float32)
    return out



# revision 56
# speedup vs baseline: 1.0094x; 1.0094x over previous
"""Trainium2 Bass kernel for causal GQA attention (B=2, T=2048, E=2048, H=16, D=128, KVH=8).

Sharding: 8 cores = 2 (batch) x 4 (head groups). Each core computes 4 query heads
(column-parallel wq) + their 2 KV heads, full causal attention for those heads, and
a partial output projection (row-parallel wo). Host sums the 4 partials per batch.

Layout strategy: everything head-transposed ([D, T] with D on partitions) so that
no on-chip transposes are needed anywhere:
  - qT/kT = wq/wk.T @ x.T directly from PE (lhsT = weight slice, rhs = x.T)
  - scores S^T[k, q] = kT.T @ qT (lhsT = kT block, rhs = qT chunk)
  - attn_outT[d, q] = v_nat.T @ expS^T (lhsT = v natural [t, d], rhs = exp block)
  - out[t, e] = attn_outT.T @ wo (lhsT = attn_outT, rhs = wo rows)
RoPE pairs are de-interleaved by permuting wq/wk columns on the host (scores are
invariant since q and k use the same permutation), so rope becomes a half-swap.
Softmax is computed without max-subtraction (logits are O(5)); causal masking is a
0/1 multiply on the exp'd diagonal blocks, full blocks above the diagonal skipped.
Denominators via ones-vector matmuls accumulated in PSUM alongside the AV matmuls.

`reps`: wraps the whole body (including input DMA) in an on-device For_i loop —
used only for latency-slope timing in test.py; the graded path uses reps=1.
"""

import numpy as np
import ml_dtypes

BF16 = ml_dtypes.bfloat16

B, T, E = 2, 2048, 2048
H, D = 16, 128
KVH = 8
THETA = 10000.0
P = 128
EO = E // P          # 16 contraction chunks
CH = 512             # q-chunk width
NTQ = T // CH        # 4 q chunks
NTB = T // P         # 16 t blocks
NH = H // 4          # 4 q heads per core
NKV = 2              # kv heads per core
SCALE = float(D) ** -0.5

_NC_CACHE = {}
_PHASE_LIMIT = "full"   # "proj" | "attn" | "full" — for phase-cost probing only
_K_STREAM = False       # False: chunk-major K projection (no bank cycling)
_EARLY_ATTN = False     # True: open the first attention block before Q's last chunk
_SROW_BATCH = True      # True: denominator matmuls as one consecutive pass per
                        # (head, chunk) instead of interleaved with AV (fewer
                        # concurrently-open PSUM accumulation groups)


def _build_nc(reps=1):
    import concourse.mybir as mybir
    import concourse.tile as tile
    from concourse import bacc

    nc = bacc.Bacc(None, target_bir_lowering=False)
    dt = mybir.dt
    f32, bf16 = dt.float32, dt.bfloat16
    Exp = mybir.ActivationFunctionType.Exp

    xT_d = nc.dram_tensor("xT", [E, T], bf16, kind="ExternalInput")
    wq_d = nc.dram_tensor("wq", [E, NH * D], bf16, kind="ExternalInput")
    wk_d = nc.dram_tensor("wk", [E, NKV * D], bf16, kind="ExternalInput")
    wv_d = nc.dram_tensor("wv", [E, NKV * D], bf16, kind="ExternalInput")
    wo_d = nc.dram_tensor("wo", [NH * D, E], bf16, kind="ExternalInput")
    cos_d = nc.dram_tensor("cosd", [P, T], bf16, kind="ExternalInput")
    sin_d = nc.dram_tensor("sind", [P, T], bf16, kind="ExternalInput")
    mk_d = nc.dram_tensor("mkd", [4, P, CH], bf16, kind="ExternalInput")
    o_d = nc.dram_tensor("od", [T, E], bf16, kind="ExternalOutput")

    xT_r = xT_d.rearrange("(eo p) t -> p eo t", p=P)
    wq_r = wq_d.rearrange("(eo p) m -> p eo m", p=P)
    wk_r = wk_d.rearrange("(eo p) m -> p eo m", p=P)
    wv_r = wv_d.rearrange("(eo p) m -> p eo m", p=P)
    wo_r = wo_d.rearrange("(h p) e -> p h e", p=P)
    o_r = o_d.rearrange("(tb p) e -> p tb e", p=P)

    with tile.TileContext(nc) as tc:
        with (
            tc.tile_pool(name="singles", bufs=1) as sg,
            tc.tile_pool(name="ropet", bufs=2) as rp,
            tc.tile_pool(name="expp", bufs=18) as ep,
            tc.tile_pool(name="normp", bufs=2) as np_,
            tc.tile_pool(name="outst", bufs=2) as op_,
        ):

            def emit_body():
                # tiles (allocation only; DMA issue order below is what matters)
                wk_sb = sg.tile([P, EO, NKV * D], bf16, name="wk_sb", tag="wk_sb")
                xT_sb = sg.tile([P, EO, T], bf16, name="xT_sb", tag="xT_sb")
                wv_sb = sg.tile([P, EO, NKV * D], bf16, name="wv_sb", tag="wv_sb")
                wq_sb = sg.tile([P, EO, NH * D], bf16, name="wq_sb", tag="wq_sb")
                cos_sb = sg.tile([P, T], bf16, name="cos_sb", tag="cos_sb")
                sin_sb = sg.tile([P, T], bf16, name="sin_sb", tag="sin_sb")
                wo_sb = sg.tile([P, NH, E], bf16, name="wo_sb", tag="wo_sb")
                mk_sb = [sg.tile([P, CH], bf16, name=f"mk{i}", tag=f"mk{i}") for i in range(4)]

                # Two parallel DMA queues, issue order = consumption order.
                # SP queue: the 8MB xT stream (eo 0 split for fast first data),
                # then late-need weights. ACT queue: small early-need weights,
                # so wk rides alongside the first xT chunks.
                # wk rides the ACT queue (first quarter alone so the first K
                # matmul starts ~0.5us in); everything else queues on SP behind
                # the xT stream in consumption order, keeping the xT cadence
                # (~1.4us/chunk) under the K-stream PE rate (~1.7us/chunk).
                nc.scalar.dma_start(wk_sb[:, 0, :], wk_r[:, 0, :])
                nc.scalar.dma_start(wk_sb[:, 1:4, :], wk_r[:, 1:4, :])
                nc.scalar.dma_start(wk_sb[:, 4:10, :], wk_r[:, 4:10, :])
                nc.scalar.dma_start(wk_sb[:, 10:16, :], wk_r[:, 10:16, :])
                for s in range(2):
                    nc.sync.dma_start(xT_sb[:, 0, CH * 2 * s:CH * 2 * (s + 1)],
                                      xT_r[:, 0, CH * 2 * s:CH * 2 * (s + 1)])
                for eo in range(1, EO):
                    nc.sync.dma_start(xT_sb[:, eo, :], xT_r[:, eo, :])
                nc.sync.dma_start(wv_sb[:], wv_r[:])
                nc.sync.dma_start(cos_sb[:], cos_d[:])
                nc.sync.dma_start(sin_sb[:], sin_d[:])
                nc.sync.dma_start(wq_sb[:], wq_r[:])
                for i in range(4):
                    nc.sync.dma_start(mk_sb[i][:], mk_d[i])
                nc.sync.dma_start(wo_sb[:], wo_r[:])

                ones_sb = sg.tile([P, 1], bf16, name="ones_sb", tag="ones_sb")
                nc.vector.memset(ones_sb[:], 1.0)

                kT_sb = [sg.tile([P, T], bf16, name=f"kT{g}", tag=f"kT{g}") for g in range(NKV)]
                qT_sb = [sg.tile([P, T], bf16, name=f"qT{h}", tag=f"qT{h}") for h in range(NH)]
                v_sb = sg.tile([P, NTB, NKV * D], bf16, name="v_sb", tag="v_sb")

                def rope_chunk(dest, sl, ps):
                    # dest[:, sl] = ps * cos + swap_halves(ps) * sin (sin rows 0:64
                    # pre-negated). ACT builds the half-swapped copy from PSUM
                    # (PSUM sources may cross partition starts; SBUF-SBUF ops may
                    # not), so the remaining DVE muls run all-SBUF 2-byte at the
                    # fast DVE rate instead of half-partition-height full-cost ops.
                    xs = rp.tile([P, CH], bf16, name="ropexs", tag="ropexs")
                    nc.scalar.copy(xs[0:64, :], ps[64:128, :])
                    nc.scalar.copy(xs[64:128, :], ps[0:64, :])
                    t1 = rp.tile([P, CH], bf16, name="ropet1", tag="ropet1")
                    nc.vector.tensor_mul(t1[:], ps[:], cos_sb[:, sl])
                    t2 = rp.tile([P, CH], bf16, name="ropet2", tag="ropet2")
                    nc.vector.tensor_mul(t2[:], xs[:], sin_sb[:, sl])
                    nc.vector.tensor_add(dest[:, sl], t1[:], t2[:])

                # Projection phase: one 8-bank PSUM ring pool for K, V and Q so
                # bank reuse follows consumption order deterministically.
                # K streams the xT chunks as the DMA delivers them: all 8
                # (kv-head, q-chunk) accumulators live in PSUM at once, one
                # matmul per accumulator per arriving eo chunk (~1.7us PE work
                # per ~1.4us DMA cadence), instead of stalling ~20us for the
                # full xT stream as with chunk-major order. V and Q then run
                # at full PE speed on the resident xT; each ring slot's next
                # user only waits for that slot's drain (ACT rope copy / v
                # copy), never for the whole previous stage.
                with tc.tile_pool(name="pp", bufs=8, space="PSUM") as pp:
                    if _K_STREAM:
                        psk = [[pp.tile([P, CH], f32, name=f"psk{g}_{t}", tag="pp")
                                for t in range(NTQ)] for g in range(NKV)]
                        for eo in range(EO):
                            for g in range(NKV):
                                for tci in range(NTQ):
                                    nc.tensor.matmul(
                                        psk[g][tci][:],
                                        wk_sb[:, eo, D * g:D * (g + 1)],
                                        xT_sb[:, eo, CH * tci:CH * (tci + 1)],
                                        start=(eo == 0), stop=(eo == EO - 1),
                                    )
                        for g in range(NKV):
                            for tci in range(NTQ):
                                rope_chunk(kT_sb[g], slice(CH * tci, CH * (tci + 1)),
                                           psk[g][tci])
                    else:
                        for g in range(NKV):
                            for tci in range(NTQ):
                                psk1 = pp.tile([P, CH], f32, name=f"psk{g}_{tci}", tag="pp")
                                for eo in range(EO):
                                    nc.tensor.matmul(
                                        psk1[:],
                                        wk_sb[:, eo, D * g:D * (g + 1)],
                                        xT_sb[:, eo, CH * tci:CH * (tci + 1)],
                                        start=(eo == 0), stop=(eo == EO - 1),
                                    )
                                rope_chunk(kT_sb[g], slice(CH * tci, CH * (tci + 1)), psk1)

                    for u in range(8):
                        psv = pp.tile([P, CH], f32, name="psv", tag="pp")
                        for k2 in range(2):
                            tb = 2 * u + k2
                            for eo in range(EO):
                                nc.tensor.matmul(
                                    psv[:, 256 * k2:256 * (k2 + 1)],
                                    xT_sb[:, eo, P * tb:P * (tb + 1)],
                                    wv_sb[:, eo, :],
                                    start=(eo == 0), stop=(eo == EO - 1),
                                )
                        for k2 in range(2):
                            nc.vector.tensor_copy(out=v_sb[:, 2 * u + k2, :], in_=psv[:, 256 * k2:256 * (k2 + 1)])

                with (
                    tc.tile_pool(name="pj", bufs=2, space="PSUM") as pj,
                    tc.tile_pool(name="ps_s", bufs=3, space="PSUM") as ps_s,
                    tc.tile_pool(name="ps_o", bufs=2, space="PSUM") as ps_o,
                    tc.tile_pool(name="ps_m", bufs=1, space="PSUM") as ps_m,
                ):
                    def attn(h, tci, norm_pieces=1):
                        # generator: yields once per k-block so the driver can
                        # interleave wo-projection steps between blocks
                        g = h // 2
                        sl = slice(CH * tci, CH * (tci + 1))
                        ntk = 4 * tci + 4
                        o_ps = ps_o.tile([P, CH], f32, name="o_ps", tag="o_ps")
                        s_row = ps_m.tile([1, CH], f32, name="s_row", tag="s_row")

                        def block_c0(j):
                            di = j - 4 * tci
                            return P * di if di > 0 else 0

                        def scores_exp(j):
                            # scores block j + its exp, emitted one block ahead of
                            # the consuming AV/denominator matmuls so PE never
                            # waits on ACT's exp latency
                            c0 = block_c0(j)
                            qsl = slice(CH * tci + c0, CH * (tci + 1))
                            s_ps = ps_s.tile([P, CH], f32, name="s_ps", tag="s_ps")
                            nc.tensor.matmul(
                                s_ps[:, c0:], kT_sb[g][:, P * j:P * (j + 1)], qT_sb[h][:, qsl],
                                start=True, stop=True,
                            )
                            e_t = ep.tile([P, CH], bf16, name="e_t", tag="e_t")
                            nc.scalar.activation(e_t[:, c0:], s_ps[:, c0:], Exp, scale=SCALE)
                            return e_t

                        # scores/exp run two blocks ahead of the consuming AV
                        # matmuls (hides exp latency); the denominator matmuls
                        # run as one consecutive same-bank pass at the end —
                        # keeping a third accumulation group open inside the j
                        # loop is expensive on hardware.
                        pend = [scores_exp(0), scores_exp(1)]
                        kept = []
                        for j in range(ntk):
                            c0 = block_c0(j)
                            e_t = pend.pop(0)
                            if j + 2 < ntk:
                                pend.append(scores_exp(j + 2))
                            if j - 4 * tci >= 0:
                                nc.vector.tensor_mul(e_t[:, c0:], e_t[:, c0:],
                                                     mk_sb[j - 4 * tci][:, c0:])
                            nc.tensor.matmul(
                                o_ps[:, c0:], v_sb[:, j, D * g:D * (g + 1)], e_t[:, c0:],
                                start=(j == 0), stop=(j == ntk - 1),
                            )
                            if _SROW_BATCH:
                                kept.append((e_t, c0))
                            else:
                                nc.tensor.matmul(
                                    s_row[:, c0:], ones_sb[:], e_t[:, c0:],
                                    start=(j == 0), stop=(j == ntk - 1),
                                )
                            yield
                        for j, (e_t, c0) in enumerate(kept):
                            nc.tensor.matmul(
                                s_row[:, c0:], ones_sb[:], e_t[:, c0:],
                                start=(j == 0), stop=(j == ntk - 1),
                            )
                        # norm_pieces>1 (very last head): normalize in t-block-sized
                        # pieces so the trailing wo strips can start on piece 0
                        # instead of waiting for the full-width chain
                        w = CH // norm_pieces
                        for pc in range(norm_pieces):
                            psl = slice(w * pc, w * (pc + 1))
                            rec = np_.tile([1, CH], f32, name="rec", tag="rec")
                            nc.vector.reciprocal(rec[:, psl], s_row[:, psl])
                            bc = np_.tile([P, CH], f32, name="bc", tag="bc")
                            nc.gpsimd.partition_broadcast(bc[:, psl], rec[:, psl])
                            nc.vector.tensor_mul(qT_sb[h][:, sl][:, psl], o_ps[:, psl], bc[:, psl])

                    if _PHASE_LIMIT == "proj":
                        for h in range(NH):
                            nc.sync.dma_start(o_r[:, 4 * h, :], qT_sb[h][:])
                        for g in range(NKV):
                            nc.sync.dma_start(o_r[:, 8 + g, :], kT_sb[g][:])
                        return

                    # The partial output projection for each chunk is interleaved
                    # into the NEXT chunk's attention at k-block granularity (one
                    # wo strip per attention block): PE alternates ~640ns
                    # attention work and ~850ns wo work while ACT runs exps
                    # ahead into the e_t ring, so neither engine stalls the
                    # other at chunk transitions.
                    def wo_group(tci, split=False):
                        # generator: yields once per 512-col strip.
                        # split=True (final t-block): DMA each 512-col strip as
                        # its copy lands so the end-of-kernel drain is one strip,
                        # not a full row
                        for tb in range(4 * tci, 4 * tci + 4):
                            ost = op_.tile([P, E], bf16, name="ost", tag="ost")
                            strips = split and tb == 4 * tci + 3
                            for n in range(4):
                                wop = pj.tile([P, CH], f32, name="wop", tag="pj")
                                for h in range(NH):
                                    nc.tensor.matmul(
                                        wop[:],
                                        qT_sb[h][:, P * tb:P * (tb + 1)],
                                        wo_sb[:, h, CH * n:CH * (n + 1)],
                                        start=(h == 0), stop=(h == NH - 1),
                                    )
                                # wo copies on DVE mid-attention (ACT is saturated
                                # by exps there; gpsimd cannot read PSUM); the
                                # final group goes to ACT, idle once exps end,
                                # while DVE still drains normalize work
                                eng = nc.scalar if split else nc.vector
                                if split:
                                    eng.copy(ost[:, CH * n:CH * (n + 1)], wop[:])
                                else:
                                    eng.tensor_copy(out=ost[:, CH * n:CH * (n + 1)], in_=wop[:])
                                if strips:
                                    nc.sync.dma_start(o_r[:, tb, CH * n:CH * (n + 1)],
                                                      ost[:, CH * n:CH * (n + 1)])
                                yield
                            if not strips:
                                nc.sync.dma_start(o_r[:, tb, :], ost[:])

                    if _PHASE_LIMIT == "attn":
                        for tci in range(NTQ):
                            for h in range(NH):
                                for _ in attn(h, tci):
                                    pass
                        for h in range(NH):
                            nc.sync.dma_start(o_r[:, 4 * h, :], qT_sb[h][:])
                        return

                    # Q projection on pj's 2-buf ring (chunk-major, eo-inner) so
                    # the attention banks (ps_s/o/m) carry no WAR against late Q
                    # rope drains when attention starts. The first attention
                    # generator's opening block is emitted just before the last
                    # Q chunk so its exp latency hides under those matmuls.
                    first_gen = None
                    for i, (h, tci) in enumerate([(h, t) for h in range(NH) for t in range(NTQ)]):
                        if _EARLY_ATTN and i == NH * NTQ - 1:
                            first_gen = attn(0, 0)
                            next(first_gen)
                        psq = pj.tile([P, CH], f32, name=f"psq{tci}", tag="pj")
                        for eo in range(EO):
                            nc.tensor.matmul(
                                psq[:],
                                wq_sb[:, eo, D * h:D * (h + 1)],
                                xT_sb[:, eo, CH * tci:CH * (tci + 1)],
                                start=(eo == 0), stop=(eo == EO - 1),
                            )
                        rope_chunk(qT_sb[h], slice(CH * tci, CH * (tci + 1)), psq)

                    done = object()
                    wo_pend = None
                    stagger = 0
                    for tci in range(NTQ):
                        if tci > 0:
                            wo_pend = wo_group(tci - 1)
                            # hold the first wo steps back a couple of blocks so
                            # the previous chunk's final normalize chain drains
                            stagger = 2
                        for h in range(NH):
                            last = tci == NTQ - 1 and h == NH - 1
                            gen = first_gen if (tci == 0 and h == 0 and first_gen is not None) else \
                                attn(h, tci, norm_pieces=4 if last else 1)
                            for _ in gen:
                                if stagger > 0:
                                    stagger -= 1
                                elif wo_pend is not None and next(wo_pend, done) is done:
                                    wo_pend = None
                    for _ in wo_group(NTQ - 1, split=True):
                        pass

            if reps > 1:
                with tc.For_i(0, reps, 1):
                    emit_body()
            else:
                emit_body()

    nc.finalize()
    return nc


def get_nc(reps=1):
    if reps not in _NC_CACHE:
        _NC_CACHE[reps] = _build_nc(reps)
    return _NC_CACHE[reps]


def make_host_inputs(x, wq, wk, wv, wo):
    """Returns per-core in_maps (list of 8 dicts)."""
    perm = np.concatenate([np.arange(0, D, 2), np.arange(1, D, 2)])
    wq4 = np.asarray(wq).reshape(E, H, D)[:, :, perm]
    wk4 = np.asarray(wk).reshape(E, KVH, D)[:, :, perm]
    wv4 = np.asarray(wv).reshape(E, KVH, D)
    wo4 = np.asarray(wo).reshape(H, D, E)
    xT = np.ascontiguousarray(np.transpose(np.asarray(x), (0, 2, 1))).astype(BF16)

    # mirror reference's float32 rope computation
    invf = 1.0 / (np.float32(THETA) ** (np.arange(0, D, 2, dtype=np.float32) / np.float32(D)))
    ang = np.arange(T, dtype=np.float32)[None, :] * invf[:, None]     # [64, T]
    cosv = np.cos(ang).astype(np.float32)
    sinv = np.sin(ang).astype(np.float32)
    cos_h = np.concatenate([cosv, cosv], 0).astype(BF16)
    sin_h = np.concatenate([-sinv, sinv], 0).astype(BF16)

    ii = np.arange(P)[:, None]
    jj = np.arange(CH)[None, :]
    mk_h = np.stack([(jj >= ii + P * di) for di in range(4)]).astype(BF16)

    in_maps = []
    for c in range(8):
        b, hg = divmod(c, 4)
        qs = slice(4 * hg, 4 * hg + 4)
        ks = slice(2 * hg, 2 * hg + 2)
        in_maps.append({
            "xT": xT[b],
            "wq": np.ascontiguousarray(wq4[:, qs].reshape(E, NH * D)).astype(BF16),
            "wk": np.ascontiguousarray(wk4[:, ks].reshape(E, NKV * D)).astype(BF16),
            "wv": np.ascontiguousarray(wv4[:, ks].reshape(E, NKV * D)).astype(BF16),
            "wo": np.ascontiguousarray(wo4[qs].reshape(NH * D, E)).astype(BF16),
            "cosd": cos_h,
            "sind": sin_h,
            "mkd": mk_h,
        })
    return in_maps


def kernel(x, mask, wq, wk, wv, wo, **extra):
    from concourse.bass_utils import run_bass_kernel_spmd

    nc = get_nc()
    in_maps = make_host_inputs(x, wq, wk, wv, wo)
    res = run_bass_kernel_spmd(nc, in_maps, core_ids=list(range(8)))
    out = np.zeros((B, T, E), np.float32)
    for c in range(8):
        out[c // 4] += res.results[c]["od"].astype(np.float32)
    return out



# revision 57
# speedup vs baseline: 1.1837x; 1.1727x over previous
"""Trainium2 Bass kernel for causal GQA attention (B=2, T=2048, E=2048, H=16, D=128, KVH=8).

Sharding: 8 cores = 2 (batch) x 4 (head groups). Each core computes 4 query heads
(column-parallel wq) + their 2 KV heads, full causal attention for those heads, and
a partial output projection (row-parallel wo). Host sums the 4 partials per batch.

Layout strategy: everything head-transposed ([D, T] with D on partitions) so that
no on-chip transposes are needed anywhere:
  - qT/kT = wq/wk.T @ x.T directly from PE (lhsT = weight slice, rhs = x.T)
  - scores S^T[k, q] = kT.T @ qT (lhsT = kT block, rhs = qT chunk)
  - attn_outT[d, q] = v_nat.T @ expS^T (lhsT = v natural [t, d], rhs = exp block)
  - out[t, e] = attn_outT.T @ wo (lhsT = attn_outT, rhs = wo rows)
RoPE pairs are de-interleaved by permuting wq/wk columns on the host (scores are
invariant since q and k use the same permutation), so rope becomes a half-swap:
two ACT copies build the swapped operand from PSUM (legal across partition
starts only for PSUM sources), and the muls/add run as all-SBUF 2-byte DVE ops
at the fast DVE rate, with cos/sin stored bf16.
Softmax is computed without max-subtraction (logits are O(5)); causal masking is a
0/1 multiply on the exp'd diagonal blocks, full blocks above the diagonal skipped.

Schedule (single-shot optimized; verified against the TimelineSim cost model):
  - Two DMA queues: wk on the ACT queue (first eo-chunk alone, so the first K
    matmul starts ~1.5us in); the 8MB xT stream + remaining weights on the SP
    queue in consumption order.
  - Projection phase on one 8-bank PSUM ring pool (K, then V) so each ring
    slot's next user waits only for that slot's drain; Q projection runs on the
    wo pool's 2-buf ring so the attention banks carry no WAR at the transition.
  - Attention: scores+exp emitted two k-blocks ahead of the consuming AV
    matmuls (hides exp latency); softmax denominators (ones-vector matmuls) run
    as one consecutive same-bank PSUM pass per (head, chunk) — keeping a third
    accumulation group open inside the j loop measurably slows real hardware.
  - Each chunk's wo projection strips are interleaved into the next chunk's
    attention at k-block granularity via generators, so ACT (exp) and PE trade
    work instead of stalling at chunk transitions.
  - Tail: the final head's normalize runs in t-block pieces and the final
    t-block's output DMA goes out strip-by-strip, shrinking the end drain.

`reps`: wraps the whole body (including input DMA) in an on-device For_i loop —
used only for latency-slope timing in test.py; the graded path uses reps=1.
"""

import numpy as np
import ml_dtypes

BF16 = ml_dtypes.bfloat16

B, T, E = 2, 2048, 2048
H, D = 16, 128
KVH = 8
THETA = 10000.0
P = 128
EO = E // P          # 16 contraction chunks
CH = 512             # q-chunk width
NTQ = T // CH        # 4 q chunks
NTB = T // P         # 16 t blocks
NH = H // 4          # 4 q heads per core
NKV = 2              # kv heads per core
SCALE = float(D) ** -0.5

_NC_CACHE = {}
_PHASE_LIMIT = "full"   # "proj" | "attn" | "full" — for phase-cost probing only
_K_STREAM = False       # False: chunk-major K projection (no bank cycling)
_EARLY_ATTN = False     # True: open the first attention block before Q's last chunk
_SROW_BATCH = True      # True: denominator matmuls as one consecutive pass per
                        # (head, chunk) instead of interleaved with AV (fewer
                        # concurrently-open PSUM accumulation groups)


def _build_nc(reps=1):
    import concourse.mybir as mybir
    import concourse.tile as tile
    from concourse import bacc

    nc = bacc.Bacc(None, target_bir_lowering=False)
    dt = mybir.dt
    f32, bf16 = dt.float32, dt.bfloat16
    Exp = mybir.ActivationFunctionType.Exp

    xT_d = nc.dram_tensor("xT", [E, T], bf16, kind="ExternalInput")
    wq_d = nc.dram_tensor("wq", [E, NH * D], bf16, kind="ExternalInput")
    wk_d = nc.dram_tensor("wk", [E, NKV * D], bf16, kind="ExternalInput")
    wv_d = nc.dram_tensor("wv", [E, NKV * D], bf16, kind="ExternalInput")
    wo_d = nc.dram_tensor("wo", [NH * D, E], bf16, kind="ExternalInput")
    cos_d = nc.dram_tensor("cosd", [P, T], bf16, kind="ExternalInput")
    sin_d = nc.dram_tensor("sind", [P, T], bf16, kind="ExternalInput")
    mk_d = nc.dram_tensor("mkd", [4, P, CH], bf16, kind="ExternalInput")
    o_d = nc.dram_tensor("od", [T, E], bf16, kind="ExternalOutput")

    xT_r = xT_d.rearrange("(eo p) t -> p eo t", p=P)
    wq_r = wq_d.rearrange("(eo p) m -> p eo m", p=P)
    wk_r = wk_d.rearrange("(eo p) m -> p eo m", p=P)
    wv_r = wv_d.rearrange("(eo p) m -> p eo m", p=P)
    wo_r = wo_d.rearrange("(h p) e -> p h e", p=P)
    o_r = o_d.rearrange("(tb p) e -> p tb e", p=P)

    with tile.TileContext(nc) as tc:
        with (
            tc.tile_pool(name="singles", bufs=1) as sg,
            tc.tile_pool(name="ropet", bufs=2) as rp,
            tc.tile_pool(name="expp", bufs=18) as ep,
            tc.tile_pool(name="normp", bufs=2) as np_,
            tc.tile_pool(name="outst", bufs=2) as op_,
        ):

            def emit_body():
                # tiles (allocation only; DMA issue order below is what matters)
                wk_sb = sg.tile([P, EO, NKV * D], bf16, name="wk_sb", tag="wk_sb")
                xT_sb = sg.tile([P, EO, T], bf16, name="xT_sb", tag="xT_sb")
                wv_sb = sg.tile([P, EO, NKV * D], bf16, name="wv_sb", tag="wv_sb")
                wq_sb = sg.tile([P, EO, NH * D], bf16, name="wq_sb", tag="wq_sb")
                cos_sb = sg.tile([P, T], bf16, name="cos_sb", tag="cos_sb")
                sin_sb = sg.tile([P, T], bf16, name="sin_sb", tag="sin_sb")
                wo_sb = sg.tile([P, NH, E], bf16, name="wo_sb", tag="wo_sb")
                mk_sb = [sg.tile([P, CH], bf16, name=f"mk{i}", tag=f"mk{i}") for i in range(4)]

                # Two parallel DMA queues, issue order = consumption order.
                # SP queue: the 8MB xT stream (eo 0 split for fast first data),
                # then late-need weights. ACT queue: small early-need weights,
                # so wk rides alongside the first xT chunks.
                # wk rides the ACT queue (first quarter alone so the first K
                # matmul starts ~0.5us in); everything else queues on SP behind
                # the xT stream in consumption order, keeping the xT cadence
                # (~1.4us/chunk) under the K-stream PE rate (~1.7us/chunk).
                nc.scalar.dma_start(wk_sb[:, 0, :], wk_r[:, 0, :])
                nc.scalar.dma_start(wk_sb[:, 1:4, :], wk_r[:, 1:4, :])
                nc.scalar.dma_start(wk_sb[:, 4:10, :], wk_r[:, 4:10, :])
                nc.scalar.dma_start(wk_sb[:, 10:16, :], wk_r[:, 10:16, :])
                for s in range(2):
                    nc.sync.dma_start(xT_sb[:, 0, CH * 2 * s:CH * 2 * (s + 1)],
                                      xT_r[:, 0, CH * 2 * s:CH * 2 * (s + 1)])
                for eo in range(1, EO):
                    nc.sync.dma_start(xT_sb[:, eo, :], xT_r[:, eo, :])
                nc.sync.dma_start(wv_sb[:], wv_r[:])
                nc.sync.dma_start(cos_sb[:], cos_d[:])
                nc.sync.dma_start(sin_sb[:], sin_d[:])
                nc.sync.dma_start(wq_sb[:], wq_r[:])
                for i in range(4):
                    nc.sync.dma_start(mk_sb[i][:], mk_d[i])
                nc.sync.dma_start(wo_sb[:], wo_r[:])

                ones_sb = sg.tile([P, 1], bf16, name="ones_sb", tag="ones_sb")
                nc.vector.memset(ones_sb[:], 1.0)

                kT_sb = [sg.tile([P, T], bf16, name=f"kT{g}", tag=f"kT{g}") for g in range(NKV)]
                qT_sb = [sg.tile([P, T], bf16, name=f"qT{h}", tag=f"qT{h}") for h in range(NH)]
                v_sb = sg.tile([P, NTB, NKV * D], bf16, name="v_sb", tag="v_sb")

                def rope_chunk(dest, sl, ps):
                    # dest[:, sl] = ps * cos + swap_halves(ps) * sin (sin rows 0:64
                    # pre-negated). ACT builds the half-swapped copy from PSUM
                    # (PSUM sources may cross partition starts; SBUF-SBUF ops may
                    # not), so the remaining DVE muls run all-SBUF 2-byte at the
                    # fast DVE rate instead of half-partition-height full-cost ops.
                    xs = rp.tile([P, CH], bf16, name="ropexs", tag="ropexs")
                    nc.scalar.copy(xs[0:64, :], ps[64:128, :])
                    nc.scalar.copy(xs[64:128, :], ps[0:64, :])
                    t1 = rp.tile([P, CH], bf16, name="ropet1", tag="ropet1")
                    nc.vector.tensor_mul(t1[:], ps[:], cos_sb[:, sl])
                    t2 = rp.tile([P, CH], bf16, name="ropet2", tag="ropet2")
                    nc.vector.tensor_mul(t2[:], xs[:], sin_sb[:, sl])
                    nc.vector.tensor_add(dest[:, sl], t1[:], t2[:])

                # Projection phase: one 8-bank PSUM ring pool for K, V and Q so
                # bank reuse follows consumption order deterministically.
                # K streams the xT chunks as the DMA delivers them: all 8
                # (kv-head, q-chunk) accumulators live in PSUM at once, one
                # matmul per accumulator per arriving eo chunk (~1.7us PE work
                # per ~1.4us DMA cadence), instead of stalling ~20us for the
                # full xT stream as with chunk-major order. V and Q then run
                # at full PE speed on the resident xT; each ring slot's next
                # user only waits for that slot's drain (ACT rope copy / v
                # copy), never for the whole previous stage.
                with tc.tile_pool(name="pp", bufs=8, space="PSUM") as pp:
                    if _K_STREAM:
                        psk = [[pp.tile([P, CH], f32, name=f"psk{g}_{t}", tag="pp")
                                for t in range(NTQ)] for g in range(NKV)]
                        for eo in range(EO):
                            for g in range(NKV):
                                for tci in range(NTQ):
                                    nc.tensor.matmul(
                                        psk[g][tci][:],
                                        wk_sb[:, eo, D * g:D * (g + 1)],
                                        xT_sb[:, eo, CH * tci:CH * (tci + 1)],
                                        start=(eo == 0), stop=(eo == EO - 1),
                                    )
                        for g in range(NKV):
                            for tci in range(NTQ):
                                rope_chunk(kT_sb[g], slice(CH * tci, CH * (tci + 1)),
                                           psk[g][tci])
                    else:
                        for g in range(NKV):
                            for tci in range(NTQ):
                                psk1 = pp.tile([P, CH], f32, name=f"psk{g}_{tci}", tag="pp")
                                for eo in range(EO):
                                    nc.tensor.matmul(
                                        psk1[:],
                                        wk_sb[:, eo, D * g:D * (g + 1)],
                                        xT_sb[:, eo, CH * tci:CH * (tci + 1)],
                                        start=(eo == 0), stop=(eo == EO - 1),
                                    )
                                rope_chunk(kT_sb[g], slice(CH * tci, CH * (tci + 1)), psk1)

                    for u in range(8):
                        psv = pp.tile([P, CH], f32, name="psv", tag="pp")
                        for k2 in range(2):
                            tb = 2 * u + k2
                            for eo in range(EO):
                                nc.tensor.matmul(
                                    psv[:, 256 * k2:256 * (k2 + 1)],
                                    xT_sb[:, eo, P * tb:P * (tb + 1)],
                                    wv_sb[:, eo, :],
                                    start=(eo == 0), stop=(eo == EO - 1),
                                )
                        for k2 in range(2):
                            nc.vector.tensor_copy(out=v_sb[:, 2 * u + k2, :], in_=psv[:, 256 * k2:256 * (k2 + 1)])

                with (
                    tc.tile_pool(name="pj", bufs=2, space="PSUM") as pj,
                    tc.tile_pool(name="ps_s", bufs=3, space="PSUM") as ps_s,
                    tc.tile_pool(name="ps_o", bufs=2, space="PSUM") as ps_o,
                    tc.tile_pool(name="ps_m", bufs=1, space="PSUM") as ps_m,
                ):
                    def attn(h, tci, norm_pieces=1):
                        # generator: yields once per k-block so the driver can
                        # interleave wo-projection steps between blocks
                        g = h // 2
                        sl = slice(CH * tci, CH * (tci + 1))
                        ntk = 4 * tci + 4
                        o_ps = ps_o.tile([P, CH], f32, name="o_ps", tag="o_ps")
                        s_row = ps_m.tile([1, CH], f32, name="s_row", tag="s_row")

                        def block_c0(j):
                            di = j - 4 * tci
                            return P * di if di > 0 else 0

                        def scores_exp(j):
                            # scores block j + its exp, emitted one block ahead of
                            # the consuming AV/denominator matmuls so PE never
                            # waits on ACT's exp latency
                            c0 = block_c0(j)
                            qsl = slice(CH * tci + c0, CH * (tci + 1))
                            s_ps = ps_s.tile([P, CH], f32, name="s_ps", tag="s_ps")
                            nc.tensor.matmul(
                                s_ps[:, c0:], kT_sb[g][:, P * j:P * (j + 1)], qT_sb[h][:, qsl],
                                start=True, stop=True,
                            )
                            e_t = ep.tile([P, CH], bf16, name="e_t", tag="e_t")
                            nc.scalar.activation(e_t[:, c0:], s_ps[:, c0:], Exp, scale=SCALE)
                            return e_t

                        # scores/exp run two blocks ahead of the consuming AV
                        # matmuls (hides exp latency); the denominator matmuls
                        # run as one consecutive same-bank pass at the end —
                        # keeping a third accumulation group open inside the j
                        # loop is expensive on hardware.
                        pend = [scores_exp(0), scores_exp(1)]
                        kept = []
                        for j in range(ntk):
                            c0 = block_c0(j)
                            e_t = pend.pop(0)
                            if j + 2 < ntk:
                                pend.append(scores_exp(j + 2))
                            if j - 4 * tci >= 0:
                                nc.vector.tensor_mul(e_t[:, c0:], e_t[:, c0:],
                                                     mk_sb[j - 4 * tci][:, c0:])
                            nc.tensor.matmul(
                                o_ps[:, c0:], v_sb[:, j, D * g:D * (g + 1)], e_t[:, c0:],
                                start=(j == 0), stop=(j == ntk - 1),
                            )
                            if _SROW_BATCH:
                                kept.append((e_t, c0))
                            else:
                                nc.tensor.matmul(
                                    s_row[:, c0:], ones_sb[:], e_t[:, c0:],
                                    start=(j == 0), stop=(j == ntk - 1),
                                )
                            yield
                        for j, (e_t, c0) in enumerate(kept):
                            nc.tensor.matmul(
                                s_row[:, c0:], ones_sb[:], e_t[:, c0:],
                                start=(j == 0), stop=(j == ntk - 1),
                            )
                        # norm_pieces>1 (very last head): normalize in t-block-sized
                        # pieces so the trailing wo strips can start on piece 0
                        # instead of waiting for the full-width chain
                        w = CH // norm_pieces
                        for pc in range(norm_pieces):
                            psl = slice(w * pc, w * (pc + 1))
                            rec = np_.tile([1, CH], f32, name="rec", tag="rec")
                            nc.vector.reciprocal(rec[:, psl], s_row[:, psl])
                            bc = np_.tile([P, CH], f32, name="bc", tag="bc")
                            nc.gpsimd.partition_broadcast(bc[:, psl], rec[:, psl])
                            nc.vector.tensor_mul(qT_sb[h][:, sl][:, psl], o_ps[:, psl], bc[:, psl])

                    if _PHASE_LIMIT == "proj":
                        for h in range(NH):
                            nc.sync.dma_start(o_r[:, 4 * h, :], qT_sb[h][:])
                        for g in range(NKV):
                            nc.sync.dma_start(o_r[:, 8 + g, :], kT_sb[g][:])
                        return

                    # The partial output projection for each chunk is interleaved
                    # into the NEXT chunk's attention at k-block granularity (one
                    # wo strip per attention block): PE alternates ~640ns
                    # attention work and ~850ns wo work while ACT runs exps
                    # ahead into the e_t ring, so neither engine stalls the
                    # other at chunk transitions.
                    def wo_group(tci, split=False):
                        # generator: yields once per 512-col strip.
                        # split=True (final t-block): DMA each 512-col strip as
                        # its copy lands so the end-of-kernel drain is one strip,
                        # not a full row
                        for tb in range(4 * tci, 4 * tci + 4):
                            ost = op_.tile([P, E], bf16, name="ost", tag="ost")
                            strips = split and tb == 4 * tci + 3
                            for n in range(4):
                                wop = pj.tile([P, CH], f32, name="wop", tag="pj")
                                for h in range(NH):
                                    nc.tensor.matmul(
                                        wop[:],
                                        qT_sb[h][:, P * tb:P * (tb + 1)],
                                        wo_sb[:, h, CH * n:CH * (n + 1)],
                                        start=(h == 0), stop=(h == NH - 1),
                                    )
                                # wo copies on DVE mid-attention (ACT is saturated
                                # by exps there; gpsimd cannot read PSUM); the
                                # final group goes to ACT, idle once exps end,
                                # while DVE still drains normalize work
                                eng = nc.scalar if split else nc.vector
                                if split:
                                    eng.copy(ost[:, CH * n:CH * (n + 1)], wop[:])
                                else:
                                    eng.tensor_copy(out=ost[:, CH * n:CH * (n + 1)], in_=wop[:])
                                if strips:
                                    nc.sync.dma_start(o_r[:, tb, CH * n:CH * (n + 1)],
                                                      ost[:, CH * n:CH * (n + 1)])
                                yield
                            if not strips:
                                nc.sync.dma_start(o_r[:, tb, :], ost[:])

                    if _PHASE_LIMIT == "attn":
                        for tci in range(NTQ):
                            for h in range(NH):
                                for _ in attn(h, tci):
                                    pass
                        for h in range(NH):
                            nc.sync.dma_start(o_r[:, 4 * h, :], qT_sb[h][:])
                        return

                    # Q projection on pj's 2-buf ring (chunk-major, eo-inner) so
                    # the attention banks (ps_s/o/m) carry no WAR against late Q
                    # rope drains when attention starts. The first attention
                    # generator's opening block is emitted just before the last
                    # Q chunk so its exp latency hides under those matmuls.
                    first_gen = None
                    for i, (h, tci) in enumerate([(h, t) for h in range(NH) for t in range(NTQ)]):
                        if _EARLY_ATTN and i == NH * NTQ - 1:
                            first_gen = attn(0, 0)
                            next(first_gen)
                        psq = pj.tile([P, CH], f32, name=f"psq{tci}", tag="pj")
                        for eo in range(EO):
                            nc.tensor.matmul(
                                psq[:],
                                wq_sb[:, eo, D * h:D * (h + 1)],
                                xT_sb[:, eo, CH * tci:CH * (tci + 1)],
                                start=(eo == 0), stop=(eo == EO - 1),
                            )
                        rope_chunk(qT_sb[h], slice(CH * tci, CH * (tci + 1)), psq)

                    done = object()
                    wo_pend = None
                    stagger = 0
                    for tci in range(NTQ):
                        if tci > 0:
                            wo_pend = wo_group(tci - 1)
                            # hold the first wo steps back a couple of blocks so
                            # the previous chunk's final normalize chain drains
                            stagger = 2
                        for h in range(NH):
                            last = tci == NTQ - 1 and h == NH - 1
                            gen = first_gen if (tci == 0 and h == 0 and first_gen is not None) else \
                                attn(h, tci, norm_pieces=4 if last else 1)
                            for _ in gen:
                                if stagger > 0:
                                    stagger -= 1
                                elif wo_pend is not None and next(wo_pend, done) is done:
                                    wo_pend = None
                    for _ in wo_group(NTQ - 1, split=True):
                        pass

            if reps > 1:
                with tc.For_i(0, reps, 1):
                    emit_body()
            else:
                emit_body()

    nc.finalize()
    return nc


def get_nc(reps=1):
    if reps not in _NC_CACHE:
        _NC_CACHE[reps] = _build_nc(reps)
    return _NC_CACHE[reps]


def make_host_inputs(x, wq, wk, wv, wo):
    """Returns per-core in_maps (list of 8 dicts)."""
    perm = np.concatenate([np.arange(0, D, 2), np.arange(1, D, 2)])
    wq4 = np.asarray(wq).reshape(E, H, D)[:, :, perm]
    wk4 = np.asarray(wk).reshape(E, KVH, D)[:, :, perm]
    wv4 = np.asarray(wv).reshape(E, KVH, D)
    wo4 = np.asarray(wo).reshape(H, D, E)
    xT = np.ascontiguousarray(np.transpose(np.asarray(x), (0, 2, 1))).astype(BF16)

    # mirror reference's float32 rope computation
    invf = 1.0 / (np.float32(THETA) ** (np.arange(0, D, 2, dtype=np.float32) / np.float32(D)))
    ang = np.arange(T, dtype=np.float32)[None, :] * invf[:, None]     # [64, T]
    cosv = np.cos(ang).astype(np.float32)
    sinv = np.sin(ang).astype(np.float32)
    cos_h = np.concatenate([cosv, cosv], 0).astype(BF16)
    sin_h = np.concatenate([-sinv, sinv], 0).astype(BF16)

    ii = np.arange(P)[:, None]
    jj = np.arange(CH)[None, :]
    mk_h = np.stack([(jj >= ii + P * di) for di in range(4)]).astype(BF16)

    in_maps = []
    for c in range(8):
        b, hg = divmod(c, 4)
        qs = slice(4 * hg, 4 * hg + 4)
        ks = slice(2 * hg, 2 * hg + 2)
        in_maps.append({
            "xT": xT[b],
            "wq": np.ascontiguousarray(wq4[:, qs].reshape(E, NH * D)).astype(BF16),
            "wk": np.ascontiguousarray(wk4[:, ks].reshape(E, NKV * D)).astype(BF16),
            "wv": np.ascontiguousarray(wv4[:, ks].reshape(E, NKV * D)).astype(BF16),
            "wo": np.ascontiguousarray(wo4[qs].reshape(NH * D, E)).astype(BF16),
            "cosd": cos_h,
            "sind": sin_h,
            "mkd": mk_h,
        })
    return in_maps


def kernel(x, mask, wq, wk, wv, wo, **extra):
    from concourse.bass_utils import run_bass_kernel_spmd

    nc = get_nc()
    in_maps = make_host_inputs(x, wq, wk, wv, wo)
    res = run_bass_kernel_spmd(nc, in_maps, core_ids=list(range(8)))
    out = np.zeros((B, T, E), np.float32)
    for c in range(8):
        out[c // 4] += res.results[c]["od"].astype(np.float32)
    return out

